# revision 1
# baseline (speedup 1.0000x reference)
"""Bahdanau attention fused kernel for Trainium2, 8-core data-parallel.

Reference computation (per batch b of 32, H=1024, S=2048):
    enc_score = encoder_out @ We + be                    [B, S, H]
    dec_score = dec @ Wd + bd                            [B, 1, H]
    score     = tanh(enc_score + dec_score)              [B, S, H]
    ls        = score @ Ws + bs                          [B, S, 1]
    w         = softmax(ls, axis=S)
    out       = sum_s w[b,s] * encoder_out[b,s,:]        [B, H]

Sharding: batch 32 -> 4 per core across 8 cores; weights replicated.
The tiny dec-score GEMM (67 MFLOP of 137 GFLOP, 0.05%) is folded into the
host-side bias preparation: bias[b] = be + bd + dec[b] @ Wd. bs is dropped
(softmax is shift-invariant). No max-subtraction in softmax: |ls| <= 16.

Per-core device layout (everything h-partitioned, prepared host-side):
    xt   [4, 4, 128, 8*512] bf16  xt[b, c, p, k*512+s'] = X[b, c*512+s', k*128+p]
    we   [128, 8*1024]      bf16  we[p, k*1024+n]       = We[k*128+p, n]
    ws   [128, 8]           bf16  ws[p, j]              = Ws[j*128+p, 0]
    bias [128, 32]          f32   bias[p, j*4+b]        = (be+bd+dec[b]@Wd)[j*128+p]
    out: ctx [4, 128, 8]    f32   ctx[b, p, j]          = out[b, j*128+p]

Device schedule per batch b (PE-bound, ~221us roofline/core at bf16):
  - enc_score.T tiles via matmul: We (stationary) x X.T (moving), 8 k-tiles
    accumulated in PSUM -> [128 h_out, 512 s]; issue cadence is the N=512
    streaming limit (~216 ns/matmul)
  - ScalarE evacuates PSUM with fused tanh(psum + bias[b,j]) -> bf16
  - ls.T = sum_j Ws[j-tile].T @ tanh-tile, accumulated in PSUM [1, 512]
  - ScalarE exp (bf16) with fused accum_out denominator (fp32)
  - ONLINE context: per s-chunk, raw exp weights are broadcast to 128
    partitions via a ones-matmul, multiplied against the cached X.T chunk
    (VectorE) and partial-reduced per k-tile (VectorE; ScalarE accum_out
    for each batch's last chunk); the softmax denominator is divided out
    once per batch. The context of chunk c is emitted after the matmuls of
    chunk c+1 so the PE never waits on the softmax chain.
"""

import numpy as np
import ml_dtypes

import concourse.tile as tile
from concourse import bacc, mybir
from concourse.bass_utils import run_bass_kernel_spmd

BF16 = mybir.dt.bfloat16
F32 = mybir.dt.float32
AF = mybir.ActivationFunctionType

N_CORES = 8
H = 1024
S = 2048
B_PER_CORE = 4
S_CHUNK = 512

# test.py can flip this to get a profiled run; the grading path never does.
PROFILE = {"trace": False, "tmpdir": None}


def build_program(b_per_core=B_PER_CORE, s=S, h=H):
    kt = h // 128
    jt = h // 128
    n_sc = s // S_CHUNK
    nc = bacc.Bacc("TRN2", target_bir_lowering=False, debug=False)

    xt_d = nc.dram_tensor(
        "xt", [b_per_core, n_sc, 128, kt * S_CHUNK], BF16, kind="ExternalInput"
    ).ap()
    we_d = nc.dram_tensor("we", [128, kt * h], BF16, kind="ExternalInput").ap()
    ws_d = nc.dram_tensor("ws", [128, jt], BF16, kind="ExternalInput").ap()
    bias_d = nc.dram_tensor(
        "bias", [128, jt * b_per_core], F32, kind="ExternalInput"
    ).ap()
    ctx_d = nc.dram_tensor("ctx", [b_per_core, 128, jt], F32, kind="ExternalOutput").ap()

    with tile.TileContext(nc) as tc:
        with (
            tc.tile_pool(name="consts", bufs=1) as consts,
            tc.tile_pool(name="xtp", bufs=12) as xtp,
            tc.tile_pool(name="scorep", bufs=10) as scorep,
            tc.tile_pool(name="smallp", bufs=2 * n_sc) as smallp,
            tc.tile_pool(name="ebcp", bufs=2 * n_sc) as ebcp,
            tc.tile_pool(name="scrp", bufs=6) as scrp,
            tc.tile_pool(name="trashp", bufs=1) as trashp,
            tc.tile_pool(name="ctxp", bufs=2) as ctxp,
            tc.tile_pool(name="ps_main", bufs=4, space="PSUM") as ps_main,
            tc.tile_pool(name="ps_ls", bufs=2, space="PSUM") as ps_ls,
            tc.tile_pool(name="ps_misc", bufs=2, space="PSUM") as ps_misc,
        ):
            # we goes FIRST on the sync ring, ahead of the xt stream: with
            # the scalar ring nearly empty, the sync ring gets all 16 SDMA
            # engines, so the first-matmul gate (we + xt[0,0]) clears at
            # full HBM bandwidth instead of splitting it with prefetch.
            we_sb = consts.tile([128, kt * h], BF16)
            nc.sync.dma_start(we_sb[:], we_d[:])
            ws_sb = consts.tile([128, jt], BF16)
            nc.scalar.dma_start(ws_sb[:], ws_d[:])
            bias_sb = consts.tile([128, jt * b_per_core], F32)
            nc.scalar.dma_start(bias_sb[:], bias_d[:])
            ones_bf = consts.tile([1, 128], BF16)
            nc.vector.memset(ones_bf[:], 1.0)
            ones_f32 = consts.tile([1, 128], F32)
            nc.vector.memset(ones_f32[:], 1.0)

            def emit_context_chunk(xt_bc, ex, ctx4_b, c, last_chunk, tail=False):
                """Broadcast chunk weights and accumulate context partials.

                The broadcast runs on the otherwise-idle GpSimd engine except
                on the kernel's final chunk, where the PE is idle and the
                ones-matmul + cast path has lower latency.
                """
                ebc = ebcp.tile([128, S_CHUNK], BF16, tag="ebc")
                if tail:
                    bc_ps = ps_misc.tile([128, S_CHUNK], F32, tag="misc")
                    nc.tensor.matmul(
                        bc_ps[:], lhsT=ones_bf[:], rhs=ex[:], start=True, stop=True
                    )
                    nc.vector.tensor_copy(ebc[:], bc_ps[:])
                else:
                    nc.gpsimd.partition_broadcast(ebc[:], ex[:])
                for k in range(kt):
                    scr = scrp.tile([128, S_CHUNK], BF16, tag="scr")
                    nc.vector.tensor_mul(
                        scr[:], xt_bc[:, k * S_CHUNK : (k + 1) * S_CHUNK], ebc[:]
                    )
                    if last_chunk and k % 2 == 0:
                        trash = trashp.tile([128, S_CHUNK], BF16, tag="trash")
                        nc.scalar.activation(
                            trash[:], scr[:], AF.Identity,
                            accum_out=ctx4_b[:, k * n_sc + c : k * n_sc + c + 1],
                        )
                    else:
                        nc.vector.reduce_sum(
                            ctx4_b[:, k * n_sc + c : k * n_sc + c + 1],
                            scr[:],
                            axis=mybir.AxisListType.X,
                        )

            def emit_invd(denom_b):
                """softmax denominator -> broadcast 1/d [128, 1]."""
                dsum = smallp.tile([1, 1], F32, tag="dsum")
                nc.vector.reduce_sum(dsum[:], denom_b[:], axis=mybir.AxisListType.X)
                invd = smallp.tile([1, 1], F32, tag="invd")
                nc.vector.reciprocal(invd[:], dsum[:])
                iv_ps = ps_misc.tile([128, S_CHUNK], F32, tag="misc")
                nc.tensor.matmul(
                    iv_ps[:, 0:1], lhsT=ones_f32[:], rhs=invd[:], start=True, stop=True
                )
                invd_bc = smallp.tile([128, 1], F32, tag="invdbc")
                nc.scalar.copy(invd_bc[:], iv_ps[:, 0:1])
                return invd_bc

            def emit_batch_final(b, ctx4_b, invd_bc):
                """Partial reduction, normalize, store."""
                ctxu = ctxp.tile([128, jt], F32, tag="ctxu")
                for k in range(kt):
                    nc.vector.reduce_sum(
                        ctxu[:, k : k + 1],
                        ctx4_b[:, k * n_sc : (k + 1) * n_sc],
                        axis=mybir.AxisListType.X,
                    )
                ctx_b = ctxp.tile([128, jt], F32, tag="ctx")
                nc.vector.tensor_scalar_mul(ctx_b[:], ctxu[:], invd_bc[:])
                nc.sync.dma_start(ctx_d[b], ctx_b[:])

            pending = []  # deferred (context-chunk | invd | batch-final)
            for b in range(b_per_core):
                xt_tiles = []
                for c in range(n_sc):
                    xt_bc = xtp.tile([128, kt * S_CHUNK], BF16, tag="xt")
                    if b == 0 and c == 0:
                        # split the gate-opening chunk so the first matmul
                        # group starts on the early half
                        half = kt // 2 * S_CHUNK
                        nc.sync.dma_start(xt_bc[:, :half], xt_d[b, c][:, :half])
                        nc.sync.dma_start(xt_bc[:, half:], xt_d[b, c][:, half:])
                    else:
                        nc.sync.dma_start(xt_bc[:], xt_d[b, c])
                    xt_tiles.append(xt_bc)

                denom_b = smallp.tile([1, n_sc], F32, tag="denom")
                ctx4_b = ctxp.tile([128, kt * n_sc], F32, tag="ctx4")
                for c in range(n_sc):
                    ls_ps = ps_ls.tile([1, S_CHUNK], F32, tag="ls")
                    score_tiles = []
                    for j in range(jt):
                        mm_ps = ps_main.tile([128, S_CHUNK], F32, tag="main")
                        for k in range(kt):
                            nc.tensor.matmul(
                                mm_ps[:],
                                lhsT=we_sb[:, k * h + j * 128 : k * h + (j + 1) * 128],
                                rhs=xt_tiles[c][:, k * S_CHUNK : (k + 1) * S_CHUNK],
                                start=(k == 0),
                                stop=(k == kt - 1),
                            )
                        sc = scorep.tile([128, S_CHUNK], BF16, tag="score")
                        nc.scalar.activation(
                            sc[:], mm_ps[:], AF.Tanh,
                            bias=bias_sb[:, j * b_per_core + b : j * b_per_core + b + 1],
                        )
                        score_tiles.append(sc)
                        if j == 0:
                            # deferred work from the previous chunk/batch is
                            # emitted right after the first matmul group, so
                            # its PE ops (weight broadcast) slot in early and
                            # the DVE context work overlaps this chunk's
                            # remaining matmul groups
                            for fn in pending:
                                fn()
                            pending = []
                    for j in range(jt):
                        nc.tensor.matmul(
                            ls_ps[:],
                            lhsT=ws_sb[:, j : j + 1],
                            rhs=score_tiles[j][:],
                            start=(j == 0),
                            stop=(j == jt - 1),
                        )
                    ex = smallp.tile([1, S_CHUNK], BF16, tag="exp")
                    nc.scalar.activation(
                        ex[:], ls_ps[:], AF.Exp, accum_out=denom_b[:, c : c + 1]
                    )

                    last_b = b == b_per_core - 1
                    ctx_fn = (
                        lambda xt_bc=xt_tiles[c], ex=ex, ctx4_b=ctx4_b, c=c,
                        lc=(c == n_sc - 1), tl=(last_b and c == n_sc - 1):
                        emit_context_chunk(xt_bc, ex, ctx4_b, c, lc, tail=tl)
                    )
                    if c < n_sc - 1:
                        pending.append(ctx_fn)
                    elif last_b:
                        # tail of the whole kernel: get 1/d going on the
                        # still-empty DVE queue, then the final context chunk
                        invd_bc = emit_invd(denom_b)
                        ctx_fn()
                        emit_batch_final(b, ctx4_b, invd_bc)
                    else:
                        def batch_tail(ctx_fn=ctx_fn, b=b, ctx4_b=ctx4_b,
                                       denom_b=denom_b):
                            invd_bc = emit_invd(denom_b)
                            ctx_fn()
                            emit_batch_final(b, ctx4_b, invd_bc)
                        pending.append(batch_tail)

    nc.compile()
    return nc


_CACHED = {}


def _get_program(key):
    if key not in _CACHED:
        _CACHED[key] = build_program(*key)
    return _CACHED[key]


def make_in_maps(encoder_out, decoder_hidden_state, We, be, Wd, bd, Ws, bs,
                 b_per_core=B_PER_CORE, s=S, h=H, n_cores=N_CORES):
    kt = h // 128
    jt = h // 128
    n_sc = s // S_CHUNK
    bf = ml_dtypes.bfloat16

    we_a = np.ascontiguousarray(
        We.reshape(kt, 128, h).transpose(1, 0, 2).reshape(128, kt * h)
    ).astype(bf)
    ws_a = np.ascontiguousarray(Ws[:, 0].reshape(jt, 128).T).astype(bf)

    dec = decoder_hidden_state[0]  # [32, h]
    bias_all = (be + bd)[None, :] + dec @ Wd  # [32, h] fp32
    in_maps = []
    for i in range(n_cores):
        b0 = i * b_per_core
        xb = encoder_out[b0 : b0 + b_per_core]  # [b, s, h]
        # [b, c, s', k, p] -> [b, c, p, k, s']
        xt_a = np.ascontiguousarray(
            xb.reshape(b_per_core, n_sc, S_CHUNK, kt, 128).transpose(0, 1, 4, 3, 2)
        ).reshape(b_per_core, n_sc, 128, kt * S_CHUNK).astype(bf)
        bias_a = np.ascontiguousarray(
            bias_all[b0 : b0 + b_per_core].reshape(b_per_core, jt, 128).transpose(2, 1, 0)
        ).reshape(128, jt * b_per_core).astype(np.float32)
        in_maps.append({"xt": xt_a, "we": we_a, "ws": ws_a, "bias": bias_a})
    return in_maps


def kernel(encoder_out, decoder_hidden_state, We, be, Wd, bd, Ws, bs):
    encoder_out = np.asarray(encoder_out, dtype=np.float32)
    decoder_hidden_state = np.asarray(decoder_hidden_state, dtype=np.float32)
    We = np.asarray(We, dtype=np.float32)
    be = np.asarray(be, dtype=np.float32)
    Wd = np.asarray(Wd, dtype=np.float32)
    bd = np.asarray(bd, dtype=np.float32)
    Ws = np.asarray(Ws, dtype=np.float32)
    bs = np.asarray(bs, dtype=np.float32)

    nc = _get_program((B_PER_CORE, S, H))
    in_maps = make_in_maps(
        encoder_out, decoder_hidden_state, We, be, Wd, bd, Ws, bs
    )
    kwargs = {}
    if PROFILE["trace"]:
        kwargs = {"trace": True, "tmpdir": PROFILE["tmpdir"]}
    res = run_bass_kernel_spmd(nc, in_maps, list(range(N_CORES)), **kwargs)
    PROFILE["last_result"] = res

    out = np.empty((N_CORES * B_PER_CORE, H), dtype=np.float32)
    for i in range(N_CORES):
        ctx = res.results[i]["ctx"]  # [b, 128, jt]
        out[i * B_PER_CORE : (i + 1) * B_PER_CORE] = (
            ctx.transpose(0, 2, 1).reshape(B_PER_CORE, H)
        )
    return out



# revision 9
# speedup vs baseline: 1.7711x; 1.7711x over previous
"""Bahdanau attention fused kernel for Trainium2, 8-core data-parallel.

Reference computation (per batch b of 32, H=1024, S=2048):
    enc_score = encoder_out @ We + be                    [B, S, H]
    dec_score = dec @ Wd + bd                            [B, 1, H]
    score     = tanh(enc_score + dec_score)              [B, S, H]
    ls        = score @ Ws + bs                          [B, S, 1]
    w         = softmax(ls, axis=S)
    out       = sum_s w[b,s] * encoder_out[b,s,:]        [B, H]

Sharding: batch 32 -> 4 per core across 8 cores; weights replicated.
The tiny dec-score GEMM is folded into the host-side bias preparation:
bias[b] = be + bd + dec[b] @ Wd. bs is dropped (softmax shift-invariant).
No max-subtraction in softmax: |ls| <= 16.

Main GEMM runs in fp8e4 DoubleRow mode (2 k-tiles per matmul, ~1.8x the
bf16 streaming rate). fp8 operands are pre-scaled (X*16, We*64) so the
uniform We values clear the fp8 subnormal threshold; the 1/1024 rescale is
folded into the tanh activation's free scale. The context accumulation
(sum_s w_s * x_s) keeps a bf16 copy of X for precision and runs as fused
multiply-reduce (scalar_tensor_tensor with accum_out) on the DVE, one
instruction per (chunk, k-tile), with per-chunk partials reduced at batch
end. (tensor_tensor_reduce would allow chaining the running sum but crashes
the DVE on this runtime.)

Per-core device layout (prepared host-side):
    xt8  [4, 4, 128, 8, 512] f8e4   xt8[b,c,p,k,s'] = 16*X[b, c*512+s', k*128+p]
    xtb  [4, 4, 128, 8*512] bf16    xtb[b,c,p,k*512+s'] = X[b, c*512+s', k*128+p]
    we8  [128, 8, 1024]     f8e4    we8[p,k,n] = 64*We[k*128+p, n]
    ws   [128, 8]           bf16    ws[p,j]    = Ws[j*128+p, 0]
    bias [128, 32]          f32     bias[p,j*4+b] = (be+bd+dec[b]@Wd)[j*128+p]
    out: ctx [4, 128, 8]    f32     ctx[b,p,k] = out[b, k*128+p]

Device schedule per (batch, chunk):
  - 8 j-groups of 4 DoubleRow matmuls (fp8, 256-contraction each) -> PSUM
  - ScalarE evacuates with fused tanh(psum/1024 + bias[b,j]) -> bf16
  - ls.T = sum_j Ws[j].T @ tanh-tile accumulated in PSUM [1, 512] (bf16 MMs)
  - ScalarE exp (bf16) with fused accum_out denominator (f32)
  - GpSimd broadcasts the raw exp weights to 128 partitions; DVE
    tensor_tensor_reduce folds x*w into per-k context partials
  - per-batch tail (1/denom broadcast via ones-matmul, gpsimd-partial
    reduction, normalize, store) is deferred into the next batch's first
    matmul-group shadow so the PE queue never waits on the softmax chain
"""

import numpy as np
import ml_dtypes

import concourse.tile as tile
from concourse import bacc, mybir
from concourse.bass_utils import run_bass_kernel_spmd

BF16 = mybir.dt.bfloat16
F32 = mybir.dt.float32
F8 = mybir.dt.float8e4
AF = mybir.ActivationFunctionType
ALU = mybir.AluOpType

N_CORES = 8
H = 1024
S = 2048
B_PER_CORE = 4
S_CHUNK = 512

X_SCALE = 16.0
WE_SCALE = 64.0
INV_SCALE = 1.0 / (X_SCALE * WE_SCALE)

# test.py can flip this to get a profiled run; the grading path never does.
PROFILE = {"trace": False, "tmpdir": None}


def build_program(b_per_core=B_PER_CORE, s=S, h=H):
    kt = h // 128
    jt = h // 128
    n_sc = s // S_CHUNK
    nc = bacc.Bacc("TRN2", target_bir_lowering=False, debug=False)

    xt8_d = nc.dram_tensor(
        "xt8", [b_per_core, n_sc, 128, kt, S_CHUNK], F8, kind="ExternalInput"
    ).ap()
    xtb_d = nc.dram_tensor(
        "xtb", [b_per_core, n_sc, 128, kt * S_CHUNK], BF16, kind="ExternalInput"
    ).ap()
    we8_d = nc.dram_tensor("we8", [128, kt, h], F8, kind="ExternalInput").ap()
    ws_d = nc.dram_tensor("ws", [128, jt], BF16, kind="ExternalInput").ap()
    bias_d = nc.dram_tensor(
        "bias", [128, jt * b_per_core], F32, kind="ExternalInput"
    ).ap()
    ctx_d = nc.dram_tensor("ctx", [b_per_core, 128, jt], F32, kind="ExternalOutput").ap()

    with tile.TileContext(nc) as tc:
        with (
            tc.tile_pool(name="consts", bufs=1) as consts,
            tc.tile_pool(name="xt8p", bufs=8) as xt8p,
            tc.tile_pool(name="xtbp", bufs=8) as xtbp,
            tc.tile_pool(name="scorep", bufs=10) as scorep,
            tc.tile_pool(name="smallp", bufs=2 * n_sc) as smallp,
            tc.tile_pool(name="ebcp", bufs=4) as ebcp,
            tc.tile_pool(name="scrp", bufs=6) as scrp,
            tc.tile_pool(name="ctxp", bufs=8) as ctxp,
            tc.tile_pool(name="ps_main", bufs=4, space="PSUM") as ps_main,
            tc.tile_pool(name="ps_ls", bufs=2, space="PSUM") as ps_ls,
            tc.tile_pool(name="ps_misc", bufs=2, space="PSUM") as ps_misc,
        ):
            # we8 goes FIRST on the sync ring, ahead of the xt8 stream: with
            # the scalar ring nearly empty, the sync ring gets all 16 SDMA
            # engines, so the first-matmul gate (we8 + xt8[0,0]) clears at
            # full HBM bandwidth instead of splitting it with prefetch.
            we8_sb = consts.tile([128, kt, h], F8)
            nc.sync.dma_start(we8_sb[:], we8_d[:])
            ws_sb = consts.tile([128, jt], BF16)
            nc.scalar.dma_start(ws_sb[:], ws_d[:])
            bias_sb = consts.tile([128, jt * b_per_core], F32)
            nc.scalar.dma_start(bias_sb[:], bias_d[:])
            ones_bf = consts.tile([1, 128], BF16)
            nc.vector.memset(ones_bf[:], 1.0)
            ones_f32 = consts.tile([1, 128], F32)
            nc.vector.memset(ones_f32[:], 1.0)

            def emit_context_chunk(xtb_bc, ex, ctx4_b, c, tail=False):
                """Broadcast chunk weights, fold x*w into context partials.

                scalar_tensor_tensor fuses the multiply and the free-axis
                sum into one DVE instruction per k-tile; per-chunk partials
                land in ctx4_b columns and are reduced at batch end. On the
                kernel's final chunk the PE is idle and the ones-matmul
                broadcast has lower latency than GpSimd.
                """
                ebc = ebcp.tile([128, S_CHUNK], BF16, tag="ebc")
                if tail:
                    bc_ps = ps_misc.tile([128, S_CHUNK], F32, tag="misc")
                    nc.tensor.matmul(
                        bc_ps[:], lhsT=ones_bf[:], rhs=ex[:], start=True, stop=True
                    )
                    nc.vector.tensor_copy(ebc[:], bc_ps[:])
                else:
                    nc.gpsimd.partition_broadcast(ebc[:], ex[:])
                for k in range(kt):
                    scr = scrp.tile([128, S_CHUNK], BF16, tag="scr")
                    nc.vector.scalar_tensor_tensor(
                        out=scr[:],
                        in0=xtb_bc[:, k * S_CHUNK : (k + 1) * S_CHUNK],
                        scalar=1.0,
                        in1=ebc[:],
                        op0=ALU.mult,
                        op1=ALU.mult,
                        accum_out=ctx4_b[:, k * n_sc + c : k * n_sc + c + 1],
                    )

            def emit_batch_final(b, denom_b, ctx4_b):
                """1/denom via ones-matmul, partial reduce, normalize, store."""
                dsum = smallp.tile([1, 1], F32, tag="dsum")
                nc.vector.reduce_sum(dsum[:], denom_b[:], axis=mybir.AxisListType.X)
                invd = smallp.tile([1, 1], F32, tag="invd")
                nc.vector.reciprocal(invd[:], dsum[:])
                iv_ps = ps_misc.tile([128, S_CHUNK], F32, tag="misc")
                nc.tensor.matmul(
                    iv_ps[:, 0:1], lhsT=ones_f32[:], rhs=invd[:], start=True, stop=True
                )
                invd_bc = smallp.tile([128, 1], F32, tag="invdbc")
                nc.scalar.copy(invd_bc[:], iv_ps[:, 0:1])
                ctxu = ctxp.tile([128, jt], F32, tag="ctxu")
                for k in range(kt):
                    nc.vector.reduce_sum(
                        ctxu[:, k : k + 1],
                        ctx4_b[:, k * n_sc : (k + 1) * n_sc],
                        axis=mybir.AxisListType.X,
                    )
                ctx_b = ctxp.tile([128, jt], F32, tag="ctx")
                nc.vector.tensor_scalar_mul(ctx_b[:], ctxu[:], invd_bc[:])
                nc.sync.dma_start(ctx_d[b], ctx_b[:])

            pending = []  # deferred per-batch tail work
            for b in range(b_per_core):
                xt8_tiles = []
                xtb_tiles = []
                for c in range(n_sc):
                    x8 = xt8p.tile([128, kt, S_CHUNK], F8, tag="xt8")
                    if b == 0 and c == 0:
                        # split the gate-opening chunk so the first matmul
                        # group starts on the early half
                        nc.sync.dma_start(x8[:, : kt // 2, :], xt8_d[b, c][:, : kt // 2, :])
                        nc.sync.dma_start(x8[:, kt // 2 :, :], xt8_d[b, c][:, kt // 2 :, :])
                    else:
                        nc.sync.dma_start(x8[:], xt8_d[b, c])
                    xt8_tiles.append(x8)
                    xb = xtbp.tile([128, kt * S_CHUNK], BF16, tag="xtb")
                    nc.scalar.dma_start(xb[:], xtb_d[b, c])
                    xtb_tiles.append(xb)

                denom_b = smallp.tile([1, n_sc], F32, tag="denom")
                ctx4_b = ctxp.tile([128, kt * n_sc], F32, tag="ctx4")

                for c in range(n_sc):
                    score_tiles = []
                    for j in range(jt):
                        mm_ps = ps_main.tile([128, S_CHUNK], F32, tag="main")
                        for kp in range(kt // 2):
                            nc.tensor.matmul(
                                mm_ps[:],
                                lhsT=we8_sb[:, 2 * kp : 2 * kp + 2, j * 128 : (j + 1) * 128],
                                rhs=xt8_tiles[c][:, 2 * kp : 2 * kp + 2, :],
                                start=(kp == 0),
                                stop=(kp == kt // 2 - 1),
                                perf_mode=mybir.MatmulPerfMode.DoubleRow,
                            )
                        sc = scorep.tile([128, S_CHUNK], BF16, tag="score")
                        nc.scalar.activation(
                            sc[:], mm_ps[:], AF.Tanh,
                            bias=bias_sb[:, j * b_per_core + b : j * b_per_core + b + 1],
                            scale=INV_SCALE,
                        )
                        score_tiles.append(sc)
                        if j == 0:
                            # deferred tail of the previous batch: its PE op
                            # (1/denom ones-matmul) slots in right after the
                            # first matmul group so the PE never waits on
                            # the softmax chain
                            for fn in pending:
                                fn()
                            pending = []
                    ls_ps = ps_ls.tile([1, S_CHUNK], F32, tag="ls")
                    for j in range(jt):
                        nc.tensor.matmul(
                            ls_ps[:],
                            lhsT=ws_sb[:, j : j + 1],
                            rhs=score_tiles[j][:],
                            start=(j == 0),
                            stop=(j == jt - 1),
                        )
                    ex = smallp.tile([1, S_CHUNK], BF16, tag="exp")
                    nc.scalar.activation(
                        ex[:], ls_ps[:], AF.Exp, accum_out=denom_b[:, c : c + 1]
                    )

                    last_b = b == b_per_core - 1
                    emit_context_chunk(
                        xtb_tiles[c], ex, ctx4_b, c,
                        tail=(last_b and c == n_sc - 1),
                    )
                    if c == n_sc - 1:
                        if last_b:
                            emit_batch_final(b, denom_b, ctx4_b)
                        else:
                            pending.append(
                                lambda b=b, denom_b=denom_b, ctx4_b=ctx4_b:
                                emit_batch_final(b, denom_b, ctx4_b)
                            )

    nc.compile()
    return nc


_CACHED = {}


def _get_program(key):
    if key not in _CACHED:
        _CACHED[key] = build_program(*key)
    return _CACHED[key]


def make_in_maps(encoder_out, decoder_hidden_state, We, be, Wd, bd, Ws, bs,
                 b_per_core=B_PER_CORE, s=S, h=H, n_cores=N_CORES):
    kt = h // 128
    jt = h // 128
    n_sc = s // S_CHUNK
    bf = ml_dtypes.bfloat16
    f8 = ml_dtypes.float8_e4m3

    we8_a = np.ascontiguousarray(
        (We * WE_SCALE).reshape(kt, 128, h).transpose(1, 0, 2)
    ).astype(f8)
    ws_a = np.ascontiguousarray(Ws[:, 0].reshape(jt, 128).T).astype(bf)

    dec = decoder_hidden_state[0]  # [32, h]
    bias_all = (be + bd)[None, :] + dec @ Wd  # [32, h] fp32
    in_maps = []
    for i in range(n_cores):
        b0 = i * b_per_core
        xb = encoder_out[b0 : b0 + b_per_core]  # [b, s, h]
        # [b, c, s', k, p] -> [b, c, p, k, s']
        xt = np.ascontiguousarray(
            xb.reshape(b_per_core, n_sc, S_CHUNK, kt, 128).transpose(0, 1, 4, 3, 2)
        )
        xt8_a = (xt * X_SCALE).astype(f8)
        xtb_a = xt.reshape(b_per_core, n_sc, 128, kt * S_CHUNK).astype(bf)
        bias_a = np.ascontiguousarray(
            bias_all[b0 : b0 + b_per_core].reshape(b_per_core, jt, 128).transpose(2, 1, 0)
        ).reshape(128, jt * b_per_core).astype(np.float32)
        in_maps.append(
            {"xt8": xt8_a, "xtb": xtb_a, "we8": we8_a, "ws": ws_a, "bias": bias_a}
        )
    return in_maps


def kernel(encoder_out, decoder_hidden_state, We, be, Wd, bd, Ws, bs):
    encoder_out = np.asarray(encoder_out, dtype=np.float32)
    decoder_hidden_state = np.asarray(decoder_hidden_state, dtype=np.float32)
    We = np.asarray(We, dtype=np.float32)
    be = np.asarray(be, dtype=np.float32)
    Wd = np.asarray(Wd, dtype=np.float32)
    bd = np.asarray(bd, dtype=np.float32)
    Ws = np.asarray(Ws, dtype=np.float32)
    bs = np.asarray(bs, dtype=np.float32)

    nc = _get_program((B_PER_CORE, S, H))
    in_maps = make_in_maps(
        encoder_out, decoder_hidden_state, We, be, Wd, bd, Ws, bs
    )
    kwargs = {}
    if PROFILE["trace"]:
        kwargs = {"trace": True, "tmpdir": PROFILE["tmpdir"]}
    res = run_bass_kernel_spmd(nc, in_maps, list(range(N_CORES)), **kwargs)
    PROFILE["last_result"] = res

    out = np.empty((N_CORES * B_PER_CORE, H), dtype=np.float32)
    for i in range(N_CORES):
        ctx = res.results[i]["ctx"]  # [b, 128, jt]
        out[i * B_PER_CORE : (i + 1) * B_PER_CORE] = (
            ctx.transpose(0, 2, 1).reshape(B_PER_CORE, H)
        )
    return out
